# revision 15
# baseline (speedup 1.0000x reference)
"""Causal multi-head self-attention on 8 Trainium2 NeuronCores.

Problem (hardcoded): x [4, 2048, 1024] fp32, w_qkv [3072, 1024], w_out
[1024, 1024], token_positions [2048] int32; H=16 heads, Dh=64, RoPE
(interleaved pairs, theta=10000), causal softmax, output projection.

Sharding: 8 cores = 4 batches x 2 head-groups (8 heads each). Each core
computes qkv projection for its heads, RoPE, causal attention, and a
partial output projection over its 512 y-features. Host sums the two
partial projections per batch and transposes back.

Fused single-pass schedule (all matmul operands fp16; PSUM stays fp32):
  prologue: DMAs, exp-table warm, PE p-state warmup, proj(chunk 0)
  iteration i in 0..3:
      emit attention(chunk i) tile stream, paced with filler units from
      proj(chunk i+1) and out-projection(chunk i-1) so the PE never
      stalls on the exp (ScalarE) dependency and keeps its p-state high
  tail: out-projection(chunk 3)

Engine placement: PE all matmuls; ScalarE exp only; DVE rope
shuffle+mul, psum copies, reciprocal, normalize muls; GpSimd rope
mul+add, causal mask mul, partition broadcast.
"""

import math

import numpy as np

import concourse.bacc as bacc
import concourse.mybir as mybir
import concourse.tile as tile
from concourse.bass_utils import run_bass_kernel_spmd

F32 = mybir.dt.float32
F16 = mybir.dt.float16

B, S, D = 4, 2048, 1024
H = 16
DH = 64
H_CORE = 8          # heads per core
N_CORES = 8
ROPE_THETA = 10000.0

CH = 512            # seq chunk (free dim of most matmuls)
N_CHUNKS = S // CH          # 4
N_STILES = S // 128         # 16
N_DTILES = D // 128         # 8
SWAP_MASK = [i ^ 1 for i in range(32)]

_EXP = mybir.ActivationFunctionType.Exp


def build_nc():
    """Build + compile the SPMD single-core program (identical on all cores)."""
    nc = bacc.Bacc("TRN2", target_bir_lowering=False, debug=False)

    xT = nc.dram_tensor("xT", [D, S], F16, kind="ExternalInput").ap()
    # [d, f] with f = [q-heads (512) | k-heads (512)] for this core's 8 heads
    wqkT = nc.dram_tensor("wqkT", [D, 2 * H_CORE * DH], F16, kind="ExternalInput").ap()
    wvT = nc.dram_tensor("wvT", [D, H_CORE * DH], F16, kind="ExternalInput").ap()
    woT = nc.dram_tensor("woT", [H_CORE * DH, D], F16, kind="ExternalInput").ap()
    cosT = nc.dram_tensor("cosT", [128, S], F32, kind="ExternalInput").ap()
    sinT = nc.dram_tensor("sinT", [128, S], F32, kind="ExternalInput").ap()
    # [tri x2]: tri[i, j] = 1 if i <= j else 0
    trimask = nc.dram_tensor("trimask", [128, 256], F16, kind="ExternalInput").ap()
    outT = nc.dram_tensor("outT", [D, S], F16, kind="ExternalOutput").ap()

    with tile.TileContext(nc) as tc:
        _build_body(nc, tc, xT, wqkT, wvT, woT, cosT, sinT, trimask, outT)
    nc.compile()
    return nc


def _build_body(nc, tc, xT, wqkT, wvT, woT, cosT, sinT, trimask, outT):
    with tc.tile_pool(name="persist", bufs=1) as persist, \
         tc.tile_pool(name="data", bufs=1) as data, \
         tc.tile_pool(name="rope", bufs=3) as rope_pool, \
         tc.tile_pool(name="exp", bufs=4) as exp_pool, \
         tc.tile_pool(name="sm", bufs=2) as sm_pool, \
         tc.tile_pool(name="p3", bufs=2) as p3_pool, \
         tc.tile_pool(name="pp", bufs=2, space="PSUM") as pp, \
         tc.tile_pool(name="pss", bufs=2, space="PSUM") as pss, \
         tc.tile_pool(name="psy", bufs=2, space="PSUM") as psy:

        cos_sb = persist.tile([128, S], F32, tag="cos")
        sin_sb = persist.tile([128, S], F32, tag="sin")
        tri_sb = persist.tile([128, 256], F16, tag="tri")

        # all 8 heads: per-(pair, chunk) tiles for q, k; 16 s-tiles of v_ext
        q_rot = [[data.tile([128, CH], F16, tag=f"q{i}_{c}", name=f"qrot{i}_{c}")
                  for c in range(N_CHUNKS)] for i in range(4)]
        k_rot = [[data.tile([128, CH], F16, tag=f"k{i}_{c}", name=f"krot{i}_{c}")
                  for c in range(N_CHUNKS)] for i in range(4)]
        v_ext = [data.tile([128, H_CORE * 65], F16, tag=f"v{i}", name=f"vext{i}")
                 for i in range(N_STILES)]
        yT = [data.tile([128, S], F16, tag=f"yT{i}", name=f"yT{i}")
              for i in range(4)]

        ones_sm = data.tile([128, H_CORE], F16, tag="ones1", name="ones_sm")
        nc.vector.memset(ones_sm[:], 1.0)
        warm = data.tile([128, H_CORE], F32, tag="warm", name="warm_sm")
        nc.scalar.activation(warm[:], ones_sm[:], _EXP, scale=1.0)
        # keep the PE busy while the first DMAs land so the p-state ramps
        junk = data.tile([128, 128], F16, tag="junk", name="junk_sm")
        nc.vector.memset(junk[:], 1.0)
        for st in range(N_STILES):
            nc.vector.tensor_copy(v_ext[st][:, 64::65], ones_sm[:])

        # ---- DMAs, spread across idle engine queues for parallel loads ----
        # first-compute deps (x0 + wv) split over sync + gpsimd queues
        wv_sb = [None] * N_DTILES
        x_chunks = {c: [None] * N_DTILES for c in range(N_CHUNKS)}
        for dt in range(N_DTILES):
            q = nc.sync if dt % 2 == 0 else nc.gpsimd
            wv = data.tile([128, 512], F16, tag=f"wv{dt}", name=f"wv{dt}")
            q.dma_start(wv[:], wvT[128 * dt:128 * (dt + 1), :])
            wv_sb[dt] = wv
            t = data.tile([128, CH], F16, tag=f"xc0_{dt}", name=f"xch0_{dt}")
            q.dma_start(t[:], xT[128 * dt:128 * (dt + 1), 0:CH])
            x_chunks[0][dt] = t
        # scalar queue: wqk (needed ~8us in), then x1, then wo
        wqk_sb = []
        for dt in range(N_DTILES):
            w = data.tile([128, 1024], F16, tag=f"wqk{dt}", name=f"wqk{dt}")
            nc.scalar.dma_start(w[:], wqkT[128 * dt:128 * (dt + 1), :])
            wqk_sb.append(w)
        # gpsimd queue: rope tables (needed ~15us in)
        nc.gpsimd.dma_start(cos_sb[:], cosT)
        nc.gpsimd.dma_start(sin_sb[:], sinT)
        nc.gpsimd.dma_start(tri_sb[:], trimask)
        for dt in range(N_DTILES):
            t = data.tile([128, CH], F16, tag=f"xc1_{dt}", name=f"xch1_{dt}")
            nc.scalar.dma_start(t[:], xT[128 * dt:128 * (dt + 1), CH:2 * CH])
            x_chunks[1][dt] = t
        wo_sb = []
        for dt in range(4):
            w = data.tile([128, D], F16, tag=f"wo{dt}", name=f"wo{dt}")
            nc.scalar.dma_start(w[:], woT[128 * dt:128 * (dt + 1), :])
            wo_sb.append(w)
        # x2/x3 trail on sync (needed at ~+45us / ~+95us)
        for c in (2, 3):
            cs = slice(CH * c, CH * (c + 1))
            for dt in range(N_DTILES):
                t = data.tile([128, CH], F16, tag=f"xc{c}_{dt}",
                              name=f"xch{c}_{dt}")
                nc.sync.dma_start(t[:], xT[128 * dt:128 * (dt + 1), cs])
                x_chunks[c][dt] = t

        # ---- PE warmup: two junk accumulation groups through the pp tag ----
        for g in range(2):
            ps_w = pp.tile([128, 128], F32, tag="pp", name=f"psw{g}")
            for i in range(32):
                nc.tensor.matmul(ps_w[:], junk[:], junk[:],
                                 start=(i == 0), stop=(i == 31))
            nc.vector.tensor_copy(warm[0:1, g:g + 1], ps_w[0:1, 0:1])

        # ---- unit builders -------------------------------------------------
        def make_proj_units(c):
            """12 filler units for chunk c: 4 v-proj groups + 8 qk+rope."""
            x_ch = x_chunks[c]
            cs = slice(CH * c, CH * (c + 1))
            units = []

            def v_unit(stl):
                st = 4 * c + stl
                ps = pp.tile([128, CH], F32, tag="pp", name=f"psv{c}_{stl}")
                for dt in range(N_DTILES):
                    nc.tensor.matmul(
                        ps[:], x_ch[dt][:, 128 * stl:128 * (stl + 1)],
                        wv_sb[dt][:],
                        start=(dt == 0), stop=(dt == N_DTILES - 1))
                out_ap = v_ext[st][:, 0:H_CORE * 65].rearrange(
                    "p (h e) -> p h e", e=65)[:, :, 0:64]
                in_ap = ps[:].rearrange("p (h e) -> p h e", e=64)
                nc.vector.tensor_copy(out_ap, in_ap)

            def qk_unit(ft):
                dest = q_rot[ft][c] if ft < 4 else k_rot[ft - 4][c]
                ps = pp.tile([128, CH], F32, tag="pp", name=f"psqk{c}_{ft}")
                for dt in range(N_DTILES):
                    nc.tensor.matmul(
                        ps[:], wqk_sb[dt][:, 128 * ft:128 * (ft + 1)],
                        x_ch[dt][:],
                        start=(dt == 0), stop=(dt == N_DTILES - 1))
                shuf = rope_pool.tile([128, CH], F32, tag="shuf", name="shuf")
                nc.vector.stream_shuffle(shuf[:], ps[:], SWAP_MASK)
                t1 = rope_pool.tile([128, CH], F32, tag="t1", name="t1")
                nc.vector.tensor_mul(t1[:], ps[:], cos_sb[:, cs])
                t2 = rope_pool.tile([128, CH], F32, tag="t2", name="t2")
                nc.gpsimd.tensor_mul(t2[:], shuf[:], sin_sb[:, cs])
                nc.vector.tensor_add(dest[:], t1[:], t2[:])

            for stl in range(4):
                units.append(lambda stl=stl: v_unit(stl))
            # q/k pair interleave so head-pair 0 is ready first
            for hp in range(4):
                units.append(lambda ft=hp: qk_unit(ft))
                units.append(lambda ft=hp + 4: qk_unit(ft))
            return units

        def make_p3_units(pc):
            """8 filler units: out projection for chunk pc."""
            def p3_unit(ot):
                ps = pp.tile([128, CH], F32, tag="pp", name=f"pso{pc}_{ot}")
                for dt in range(4):
                    nc.tensor.matmul(
                        ps[:], wo_sb[dt][:, 128 * ot:128 * (ot + 1)],
                        yT[dt][:, CH * pc:CH * (pc + 1)],
                        start=(dt == 0), stop=(dt == 3))
                osb = p3_pool.tile([128, CH], F16, tag="osb", name="osb")
                nc.vector.tensor_copy(osb[:], ps[:])
                nc.sync.dma_start(
                    outT[128 * ot:128 * (ot + 1), CH * pc:CH * (pc + 1)],
                    osb[:])
            return [lambda ot=ot: p3_unit(ot) for ot in range(8)]

        # ---- prologue: chunk 0 + chunk 1 projections straight-line ---------
        # (the PE covers the DMA-landing window; both chunks' rope tails
        # clear the DVE/GpSimd queues before attention needs them)
        for u in make_proj_units(0):
            u()
        for u in make_proj_units(1):
            u()

        # ---- iterations: attention(i) paced with proj(i+2) + p3(i-1) -------
        for i in range(N_CHUNKS):
            proj_units = make_proj_units(i + 2) if i + 2 < N_CHUNKS else []
            p3_units = make_p3_units(i - 1) if i >= 1 else []

            n_tiles = 4 * (4 * i + 4)
            n_proj = len(proj_units)
            n_p3 = len(p3_units)
            p3_from = n_tiles // 4      # p3 waits for its yT deps to drain
            emitted_proj = 0
            emitted_p3 = 0
            tile_no = 0
            c = i

            def pace():
                nonlocal emitted_proj, emitted_p3
                # proj front-loaded 2/tile; p3 spread over the remainder
                want = min(n_proj, 2 * tile_no)
                while emitted_proj < want:
                    proj_units[emitted_proj]()
                    emitted_proj += 1
                if tile_no >= p3_from and n_tiles > p3_from:
                    want = n_p3 * (tile_no - p3_from) // (n_tiles - p3_from)
                    while emitted_p3 < min(want, n_p3):
                        p3_units[emitted_p3]()
                        emitted_p3 += 1

            for hp in range(4):
                pv0 = psy.tile([65, CH], F32, tag="pv", name=f"pv0_{c}_{hp}")
                pv1 = psy.tile([65, CH], F32, tag="pv", name=f"pv1_{c}_{hp}")
                nt = 4 * c + 4
                prev = None

                def emit_pv(t, et, coff, r, stop):
                    if r >= 0:
                        # zero the upper triangle of the diagonal block
                        # (emitted one tile late so exp(t) is surely done
                        # and the DVE queue never blocks on ScalarE)
                        dg = et[:].rearrange("p (b n) -> p b n", b=2)[
                            :, :, coff:coff + 128]
                        nc.vector.tensor_mul(
                            dg, dg,
                            tri_sb[:, 0:256].rearrange("p (b n) -> p b n", b=2))
                    for hl, pv in ((0, pv0), (1, pv1)):
                        hcol = (2 * hp + hl) * 65
                        nc.tensor.matmul(
                            pv[:, coff:CH],
                            v_ext[t][:, hcol:hcol + 65],
                            et[:, CH * hl + coff:CH * hl + CH],
                            start=(t == 0), stop=stop)

                for t in range(nt):
                    r = t - 4 * c
                    coff = 128 * r if r > 0 else 0
                    ps_s = pss.tile([128, 2 * CH], F32, tag="ps_s",
                                    name="ps_s")
                    kt = k_rot[hp][t // 4]
                    ks = slice(128 * (t % 4), 128 * (t % 4 + 1))
                    qt = q_rot[hp][c]
                    qs = slice(coff, CH)
                    nc.tensor.matmul(ps_s[:, coff:CH],
                                     kt[0:64, ks], qt[0:64, qs],
                                     start=True, stop=True)
                    nc.tensor.matmul(ps_s[:, CH + coff:2 * CH],
                                     kt[64:128, ks], qt[64:128, qs],
                                     start=True, stop=True)
                    et = exp_pool.tile([128, 2 * CH], F16, tag="et", name="et")
                    src = ps_s[:].rearrange("p (b n) -> p b n", b=2)[:, :, coff:CH]
                    dst = et[:].rearrange("p (b n) -> p b n", b=2)[:, :, coff:CH]
                    nc.scalar.activation(dst, src, _EXP,
                                         scale=1.0 / math.sqrt(DH))
                    if prev is not None:
                        emit_pv(*prev, stop=False)
                    prev = (t, et, coff, r)
                    tile_no += 1
                    pace()
                emit_pv(*prev, stop=True)

                # normalize: row 64 of pv = softmax denominators; read the
                # psum directly (no staging copies) to keep DVE latency low
                for hl, pv in ((0, pv0), (1, pv1)):
                    # stage denominators to SBUF with a regular copy: the
                    # custom-DVE reciprocal must not read PSUM directly (its
                    # cross-engine wait on the closing matmul is unreliable
                    # on HW; the copy carries the dep, in-order DVE does the
                    # rest)
                    sm = sm_pool.tile([1, CH], F32, tag="sm", name="sm")
                    nc.vector.tensor_copy(sm[:], pv[64:65, :])
                    rc = sm_pool.tile([1, CH], F32, tag="rc", name="rc")
                    nc.vector.reciprocal_approx_fast(rc[:], sm[:])
                    bc = sm_pool.tile([64, CH], F32, tag="bc", name="bc")
                    nc.gpsimd.partition_broadcast(bc[:], rc[:])
                    nc.vector.tensor_mul(
                        yT[hp][64 * hl:64 * (hl + 1), CH * c:CH * (c + 1)],
                        pv[0:64, :], bc[:])
            # drain leftovers
            while emitted_proj < n_proj:
                proj_units[emitted_proj]()
                emitted_proj += 1
            while emitted_p3 < n_p3:
                p3_units[emitted_p3]()
                emitted_p3 += 1

        # ---- tail: out projection for the last chunk -----------------------
        for u in make_p3_units(N_CHUNKS - 1):
            u()


# ---------------------------------------------------------------------------
# Host side
# ---------------------------------------------------------------------------

_NC_CACHE = None


def _get_nc():
    global _NC_CACHE
    if _NC_CACHE is None:
        _NC_CACHE = build_nc()
    return _NC_CACHE


def _host_prep(x, w_qkv, w_out, token_positions):
    """Build the 8 per-core input maps."""
    x = np.ascontiguousarray(np.asarray(x, dtype=np.float32))
    w_qkv = np.asarray(w_qkv, dtype=np.float32)
    w_out = np.asarray(w_out, dtype=np.float32)
    pos = np.asarray(token_positions).astype(np.float32)

    half = DH // 2
    inv_freq = (1.0 / (ROPE_THETA ** (np.arange(half, dtype=np.float32) * (2.0 / DH))))
    ang = pos[:, None] * inv_freq[None, :]          # [S, 32]
    cos = np.cos(ang).astype(np.float32)            # [S, 32]
    sin = np.sin(ang).astype(np.float32)
    # [Dh, S] interleaved-pair layout, duplicated for 2 heads per tile
    cos64 = np.repeat(cos.T, 2, axis=0)             # [64, S]
    sin64 = np.repeat(sin.T, 2, axis=0)
    sgn = np.where(np.arange(DH) % 2 == 0, -1.0, 1.0).astype(np.float32)
    sinp = sin64 * sgn[:, None]
    cosT = np.ascontiguousarray(np.tile(cos64, (2, 1)))      # [128, S]
    sinT = np.ascontiguousarray(np.tile(sinp, (2, 1)))

    tri = np.triu(np.ones((128, 128), dtype=np.float16))     # keep i <= j
    trimask = np.ascontiguousarray(np.concatenate([tri, tri], axis=1))

    wq, wk, wv = w_qkv[0:D], w_qkv[D:2 * D], w_qkv[2 * D:3 * D]

    in_maps = []
    for core in range(N_CORES):
        b, g = divmod(core, 2)
        rows = slice(512 * g, 512 * (g + 1))
        wqkT = np.ascontiguousarray(
            np.concatenate([wq[rows], wk[rows]], axis=0).T.astype(np.float16))
        wvT = np.ascontiguousarray(wv[rows].T.astype(np.float16))
        woT = np.ascontiguousarray(w_out[:, rows].T.astype(np.float16))
        xT = np.ascontiguousarray(x[b].T.astype(np.float16))
        in_maps.append({
            "xT": xT, "wqkT": wqkT, "wvT": wvT, "woT": woT,
            "cosT": cosT, "sinT": sinT, "trimask": trimask,
        })
    return in_maps


def _gather(results):
    out = np.empty((B, S, D), dtype=np.float32)
    for b in range(B):
        acc = (results[2 * b]["outT"].astype(np.float32)
               + results[2 * b + 1]["outT"].astype(np.float32))   # [D, S]
        out[b] = acc.T
    return out


def kernel(x, w_qkv, w_out, token_positions, _trace=False, _trace_kwargs=None):
    nc = _get_nc()
    in_maps = _host_prep(x, w_qkv, w_out, token_positions)
    kw = {}
    if _trace:
        kw["trace"] = True
        kw.update(_trace_kwargs or {})
    res = run_bass_kernel_spmd(nc, in_maps, list(range(N_CORES)), **kw)
    out = _gather(res.results)
    if _trace:
        return out, res
    return out
